# revision 16
# baseline (speedup 1.0000x reference)
"""BitLinear (BitNet-style ternary-weight linear) Trainium2 kernel.

Computes, for input x [T, I], weight w [O, I], scalar scales ws, xs:
    w_q = clip(round(w / ws), -1, 1)
    x_q = clip(round(x / xs), -128, 127)
    out = (x_q @ w_q.T) * (xs * ws)          # [T, O] fp32

Strategy (8 NeuronCores, data-parallel over the token dim):
  - Each core owns T/8 = 1024 tokens; the weight is replicated.
  - Host side only reshards/relayouts: x and w are sliced and transposed so
    the contraction dim (I) lands on SBUF partitions (xT [I, T/8], wT [I, O]).
  - On device, quantization is done with the exact round-half-to-even
    "magic number" trick on the vector engine:  RN(v + 1.5*2^23) - 1.5*2^23
    rounds v to the nearest integer (ties to even) for |v| < 2^22, and the
    clip is applied in the shifted domain so each tensor needs only two
    fused tensor_scalar passes.
  - The matmul runs in bf16.  This is EXACT: quantized activations are
    integers in [-128, 127] and weights are ternary, both exactly
    representable in bf16, and PSUM accumulates in fp32 where every partial
    sum is an integer of magnitude <= 4096*128 = 2^19 < 2^24.
  - All 8 PSUM banks hold one [128, 512] accumulator per token tile, and
    the 32-step contraction is issued in k-chunks of 8 across the 8 token
    tiles, so the tensor engine starts ~3us into the kernel and tracks the
    quantization stream instead of waiting for it.  Weight quantization for
    output-block ob+1 is emitted ahead of block ob's matmuls (software
    pipelining); PSUM is drained with the (xs*ws) scale fused, alternating
    scalar/vector engines, and outputs stream back to HBM.

The scalar scales are read on the host and baked into the traced program as
immediates (the program is cached per distinct scale value), so the device
program has just two DRAM inputs and one output.

Measured on 8 axon-attached TRN2 NeuronCores: ~476 us HW exec per core
(tensor engine busy ~450 us at 220 ns per N=512 bf16 matmul ~ the 437 us
single-column-per-cycle systolic floor), output bit-exact vs the fp32 jax
reference (every accumulation is exact integer arithmetic).
"""

import sys

if "/opt/trn_rl_repo" not in sys.path:
    sys.path.insert(0, "/opt/trn_rl_repo")

import numpy as np
from contextlib import ExitStack

N_CORES = 8
TG = 4    # token groups (2D sharding: TG x OG == N_CORES)
OG = 2    # out-feature groups
P = 128
OB = 512  # output-feature block width (one PSUM bank of fp32)
MAGIC = 12582912.0  # 1.5 * 2**23: fp32 round-to-nearest-even shifter

# module-level handle for test harnesses: last BassKernelResults
last_run = None

_program_cache = {}


def _build_program(t_per, in_f, out_f, ws, xs, kc=8, xbufs=6, wbufs=12, fine_first=False,
                   coarse_after=None, split_last_drain=False, x_needs_clip=True,
                   w_pass1_gpsimd=False, use_fp8=False):
    """Build (and finalize) the single-core SPMD Bass program."""
    import concourse.bass as bass
    import concourse.mybir as mybir
    import concourse.tile as tile
    from concourse import bacc

    fp32 = mybir.dt.float32
    bf16 = mybir.dt.bfloat16
    fp8 = mybir.dt.float8e4
    qdt = fp8 if use_fp8 else bf16
    dr_mode = mybir.MatmulPerfMode.DoubleRow
    mult = mybir.AluOpType.mult
    add = mybir.AluOpType.add
    sub = mybir.AluOpType.subtract
    amin = mybir.AluOpType.min
    amax = mybir.AluOpType.max

    KT = in_f // P       # k (contraction) tiles
    NOB = out_f // OB    # output-feature blocks
    NTT = t_per // P     # token tiles
    TTW = min(NTT, 8)    # token tiles per PSUM wave (8 banks)
    NWV = (NTT + TTW - 1) // TTW
    XH = 2 if t_per >= 2048 else 1   # x-quant halves (early wave-0 availability)
    if use_fp8:
        assert KT % 2 == 0 and kc % 2 == 0

    simple = (ws == 1.0) and (xs == 1.0)
    inv_ws = 1.0 / ws
    inv_xs = 1.0 / xs
    out_scale = float(np.float32(np.float32(ws) * np.float32(xs)))

    # Bacc (not raw Bass): its finalize pipeline runs
    # generate_event_semaphores, which splits multi-wait instructions to
    # satisfy the TRN2 1-wait-per-instruction constraint walrus enforces.
    nc = bacc.Bacc()
    xT_d = nc.declare_dram_parameter("xT", [in_f, t_per], fp32, isOutput=False)
    wT_d = nc.declare_dram_parameter("wT", [in_f, out_f], fp32, isOutput=False)
    out_d = nc.declare_dram_parameter("out", [t_per, out_f], fp32, isOutput=True)

    KC = kc                     # k-tiles per PE chunk
    NCH = (KT + KC - 1) // KC   # chunks per psum accumulation group

    with ExitStack() as ctx:
        tc = ctx.enter_context(tile.TileContext(nc))
        xstage = ctx.enter_context(tc.tile_pool(name="xstage", bufs=xbufs))
        wstage = ctx.enter_context(tc.tile_pool(name="wstage", bufs=wbufs))
        xqp = ctx.enter_context(tc.tile_pool(name="xq", bufs=1))
        wqp = ctx.enter_context(tc.tile_pool(name="wq", bufs=2))
        outp = ctx.enter_context(tc.tile_pool(name="outsb", bufs=4))
        # all 8 banks: one accumulator per token tile of the current wave,
        # live across an entire output block so PE can start after the
        # first k-chunk
        psump = ctx.enter_context(tc.tile_pool(name="psum", bufs=TTW, space="PSUM"))

        xq = xqp.tile([P, KT, t_per], qdt)

        def emit_xq(k, h):
            hw = t_per // XH
            hs = slice(h * hw, (h + 1) * hw)
            st = xstage.tile([P, hw], fp32)
            nc.sync.dma_start(st[:], xT_d[k * P : (k + 1) * P, hs])
            if simple and not x_needs_clip:
                # host verified |x/xs| < 127, so the clip is a no-op and the
                # whole quantization is one fused round: (x + C) - C
                nc.vector.tensor_scalar(xq[:, k, hs], st[:], MAGIC, MAGIC, add, sub)
                return
            if simple:
                nc.vector.tensor_scalar(st[:], st[:], MAGIC, MAGIC + 127.0, add, amin)
            else:
                nc.vector.tensor_scalar(st[:], st[:], inv_xs, MAGIC, mult, add)
                nc.vector.tensor_scalar(st[:], st[:], MAGIC + 127.0, None, amin)
            nc.vector.tensor_scalar(xq[:, k, hs], st[:], MAGIC - 128.0, MAGIC, amax, sub)

        def emit_wq(wq, ob, k):
            wt = wstage.tile([P, OB], fp32)
            nc.sync.dma_start(
                wt[:], wT_d[k * P : (k + 1) * P, ob * OB : (ob + 1) * OB]
            )
            if simple:
                eng = nc.gpsimd if w_pass1_gpsimd else nc.vector
                eng.tensor_scalar(wt[:], wt[:], MAGIC, MAGIC + 1.0, add, amin)
            else:
                nc.vector.tensor_scalar(wt[:], wt[:], inv_ws, MAGIC, mult, add)
                nc.vector.tensor_scalar(wt[:], wt[:], MAGIC + 1.0, None, amin)
            nc.vector.tensor_scalar(wq[:, k, :], wt[:], MAGIC - 1.0, MAGIC, amax, sub)

        def emit_mm(pss_tt, wq, tt, klo, khi):
            # [klo, khi) in k-tile units; fp8 uses DoubleRow over k-tile pairs
            if use_fp8:
                for kk in range(klo // 2, khi // 2):
                    nc.tensor.matmul(
                        pss_tt[:],
                        xq[:, 2 * kk : 2 * kk + 2, tt * P : (tt + 1) * P],
                        wq[:, 2 * kk : 2 * kk + 2, :],
                        start=(kk == 0),
                        stop=(kk == KT // 2 - 1),
                        perf_mode=dr_mode,
                    )
            else:
                for k in range(klo, khi):
                    nc.tensor.matmul(
                        pss_tt[:],
                        xq[:, k, tt * P : (tt + 1) * P],
                        wq[:, k, :],
                        start=(k == 0),
                        stop=(k == KT - 1),
                    )

        # prologue: first token-half of x and first w block, interleaved per
        # k-tile so the first PE chunk's dependencies complete early
        wq_tiles = [wqp.tile([P, KT, OB], qdt, name="wq0", tag="wq")]
        for k in range(KT):
            emit_xq(k, 0)
            emit_wq(wq_tiles[0], 0, k)
        # later token-halves stream in behind the prologue; they are first
        # needed by wave 1 of block 0, which starts one full wave (~30us) in
        for h in range(1, XH):
            for k in range(KT):
                emit_xq(k, h)

        for ob in range(NOB):
            wq = wq_tiles[ob]
            # software pipeline: stage the NEXT block's quant ops ahead of
            # this block's matmuls in the DVE/DMA queues
            if ob + 1 < NOB:
                wq_tiles.append(wqp.tile([P, KT, OB], qdt, name=f"wq{ob+1}", tag="wq"))
                for k in range(KT):
                    emit_wq(wq_tiles[ob + 1], ob + 1, k)

            # finer chunks at the very start so PE can begin as soon as the
            # first few quantized slices land
            if fine_first and ob == 0 and KT % KC == 0 and KC >= 4:
                bounds = [0, KC // 2, KC] + [ (c + 1) * KC for c in range(1, NCH)]
            elif coarse_after is not None and ob >= coarse_after:
                # quant pipeline is far ahead by now; run each accumulation
                # group straight through (fewer psum-group re-entries)
                bounds = [0, KT]
            else:
                bounds = [c * KC for c in range(NCH + 1)]

            for wv in range(NWV):
                tts = range(wv * TTW, min((wv + 1) * TTW, NTT))
                pss = {
                    tt: psump.tile([P, OB], fp32, name=f"ps{ob}_{tt}", tag="ps")
                    for tt in tts
                }
                for ch in range(len(bounds) - 1):
                    for tt in tts:
                        emit_mm(pss[tt], wq, tt, bounds[ch], min(bounds[ch + 1], KT))
                last = ob == NOB - 1 and wv == NWV - 1
                for tt in tts:
                    ot = outp.tile([P, OB], fp32, name=f"ot{ob}_{tt}", tag="ot")
                    if split_last_drain and last:
                        # final wave: halve each drain across both engines and
                        # split the store so the kernel tail exposes less
                        H = OB // 2
                        nc.scalar.mul(ot[:, :H], pss[tt][:, :H], out_scale)
                        nc.vector.tensor_scalar(
                            ot[:, H:], pss[tt][:, H:], out_scale, None, mult
                        )
                        nc.sync.dma_start(
                            out_d[tt * P : (tt + 1) * P, ob * OB : ob * OB + H],
                            ot[:, :H],
                        )
                        nc.sync.dma_start(
                            out_d[tt * P : (tt + 1) * P, ob * OB + H : (ob + 1) * OB],
                            ot[:, H:],
                        )
                    else:
                        # alternate drain engines so drains overlap
                        if tt % 2 == 0:
                            nc.scalar.mul(ot[:], pss[tt][:], out_scale)
                        else:
                            nc.vector.tensor_scalar(
                                ot[:], pss[tt][:], out_scale, None, mult
                            )
                        nc.sync.dma_start(
                            out_d[tt * P : (tt + 1) * P, ob * OB : (ob + 1) * OB],
                            ot[:],
                        )

    if not nc.is_finalized():
        nc.finalize()
    return nc


def _get_program(t_per, in_f, out_f, ws, xs, x_needs_clip, use_fp8):
    key = (t_per, in_f, out_f, float(ws), float(xs), bool(x_needs_clip), bool(use_fp8))
    if key not in _program_cache:
        _program_cache[key] = _build_program(
            t_per, in_f, out_f, ws, xs,
            coarse_after=2, split_last_drain=True, x_needs_clip=x_needs_clip,
            use_fp8=use_fp8, w_pass1_gpsimd=False,
        )
    return _program_cache[key]


def kernel(input, weight, weight_scale, input_scale, _trace=False):
    global last_run
    from concourse.bass_utils import run_bass_kernel_spmd

    x = np.asarray(input, dtype=np.float32)
    w = np.asarray(weight, dtype=np.float32)
    ws = float(np.asarray(weight_scale).reshape(-1)[0])
    xs = float(np.asarray(input_scale).reshape(-1)[0])

    T, I = x.shape
    O = w.shape[0]
    assert w.shape[1] == I
    assert T % (TG * P) == 0 and I % P == 0 and O % (OG * OB) == 0

    t_per = T // TG      # 2D sharding: TG token groups x OG out-feature groups
    out_w = O // OG
    # If the host can prove |x| never reaches the +-127.5 rounding boundary,
    # the int8-range clip is a no-op and x-quant needs only one fused op.
    # (Always true for randn inputs; the general program handles the rest.)
    xmax = float(np.abs(x).max())
    x_needs_clip = not (ws == 1.0 and xs == 1.0 and xmax < 127.0)
    # fp8e4 holds every integer of magnitude <= 16 exactly (and ternary
    # weights exactly), so when quantized activations stay in that range the
    # DoubleRow fp8 matmul is bit-identical to the fp32 reference; otherwise
    # fall back to the (also exact) bf16 program.
    use_fp8 = xs != 0.0 and xmax / abs(xs) < 16.49
    nc = _get_program(t_per, I, out_w, ws, xs, x_needs_clip, use_fp8)

    # Host-side resharding/relayout: contraction dim onto partitions.
    # Core c owns token group c//OG and out-feature group c%OG.
    xT = np.ascontiguousarray(x.T)  # [I, T]
    wT = np.ascontiguousarray(w.T)  # [I, O]
    in_maps = [
        {
            "xT": np.ascontiguousarray(
                xT[:, (c // OG) * t_per : (c // OG + 1) * t_per]
            ),
            "wT": np.ascontiguousarray(
                wT[:, (c % OG) * out_w : (c % OG + 1) * out_w]
            ),
        }
        for c in range(N_CORES)
    ]

    if _trace:
        # tracing needs the NTFF hook (dev harness installs it); never let
        # a missing profiling stack break a plain run
        try:
            from antenv.axon_hooks import get_axon_ntff_profile_hook  # noqa: F401
        except ImportError:
            _trace = False
    res = run_bass_kernel_spmd(nc, in_maps, list(range(N_CORES)), trace=_trace)
    last_run = res
    out = np.empty((T, O), dtype=np.float32)
    for c in range(N_CORES):
        tg, og = c // OG, c % OG
        out[tg * t_per : (tg + 1) * t_per, og * out_w : (og + 1) * out_w] = (
            res.results[c]["out"]
        )
    return out

